# revision 1
# baseline (speedup 1.0000x reference)
"""Trainium2 Bass kernel for nn_Attention_62715112456978.

The reference attention is algebraically rank-1: keys/queries/values are
outer products x ⊗ w, so

    dot[b,q,k]   = c_b * x[b,q] * x[b,k],   c_b = sum_e wq*wk / sqrt(e)
    softmax-out  = m[b,q] * wv[b,:],        m[b,q] = sum_k A[b,q,k]*x[b,k]
    final        = elu(m[b,q] * r_b + v[b,q]),  r_b = sum_e wv*wo

with wq/wk/wv/wo = |state @ W.T + b| (only the products c, r are needed,
and |a|*|b| = |a*b|, so the abs never has to be materialized).

Sharding: pure data parallel over batch; 8 cores x 512 batches each.
Weights (four 128x128 + V) are tiny and replicated to every core.
"""

import numpy as np

import concourse.bacc as bacc
import concourse.bass as bass
import concourse.tile as tile
from concourse import mybir
from concourse.bass_utils import run_bass_kernel_spmd

F32 = mybir.dt.float32

N_CORES = 8
B_FULL = 4096
BC = B_FULL // N_CORES  # 512 batches per core
CH = 128                # batches per chunk (partition dim)
NCH = BC // CH          # 4 chunks per core
T = 64                  # sequence length
D = 128                 # d_state
NW = 5 * 128 - 64       # 576 = wk|wq|wv|wo (128 each) + V (64) output columns
RSQRT_E = float(1.0 / np.sqrt(128.0))
C_SHIFT = 40.0          # global exp shift; cancels in softmax, avoids overflow

_compiled = {}


def _build():
    nc = bacc.Bacc("TRN2", target_bir_lowering=False, debug=False,
                   num_devices=N_CORES)
    xd = nc.dram_tensor("x", [BC, T], F32, kind="ExternalInput")
    sd = nc.dram_tensor("state", [BC, D], F32, kind="ExternalInput")
    wd = nc.dram_tensor("wcatT", [D, NW], F32, kind="ExternalInput")
    bd = nc.dram_tensor("biascat", [1, NW], F32, kind="ExternalInput")
    od = nc.dram_tensor("out", [BC, T], F32, kind="ExternalOutput")

    with tile.TileContext(nc) as tc:
        with (
            tc.tile_pool(name="const", bufs=1) as cpool,
            tc.tile_pool(name="io", bufs=2) as iop,
            tc.tile_pool(name="big", bufs=2) as bigp,
            tc.tile_pool(name="small", bufs=2) as smp,
            tc.tile_pool(name="psum", bufs=2, space="PSUM") as psp,
            tc.tile_pool(name="psum_h", bufs=2, space="PSUM") as psh,
        ):
            # one-time constants
            wcat = cpool.tile([D, NW], F32)
            nc.sync.dma_start(wcat[:], wd[:])
            biascat = cpool.tile([1, NW], F32)
            nc.sync.dma_start(biascat[:], bd[:])
            ones_row = cpool.tile([1, CH], F32)
            nc.gpsimd.memset(ones_row[:], 1.0)
            # identity for PE transpose
            col_i = cpool.tile([128, 128], mybir.dt.int32)
            nc.gpsimd.iota(col_i[:], pattern=[[1, 128]], base=0,
                           channel_multiplier=0)
            row_i = cpool.tile([128, 128], mybir.dt.int32)
            nc.gpsimd.iota(row_i[:], pattern=[[0, 128]], base=0,
                           channel_multiplier=1)
            ident = cpool.tile([128, 128], F32)
            nc.vector.tensor_tensor(ident[:], col_i[:], row_i[:],
                                    op=mybir.AluOpType.is_equal)
            shift = cpool.tile([128, 1], F32)
            nc.gpsimd.memset(shift[:], -C_SHIFT)

            for ci in range(NCH):
                bs = ci * CH
                x_sb = iop.tile([CH, T], F32, tag="x")
                nc.sync.dma_start(x_sb[:], xd[bs:bs + CH, :])
                st_sb = iop.tile([CH, D], F32, tag="st")
                nc.sync.dma_start(st_sb[:], sd[bs:bs + CH, :])

                # stateT via PE transpose (for the hypernet matmuls)
                stT_ps = psp.tile([D, CH], F32, tag="stT")
                nc.tensor.transpose(stT_ps[:], st_sb[:], ident[:])
                stT = smp.tile([D, CH], F32, tag="stTsb")
                nc.scalar.copy(stT[:], stT_ps[:])

                # hypernet: out[b, e] = state @ W.T + bias  (bias prefilled
                # via a K=1 ones-matmul, weights accumulate on top)
                hy0 = psh.tile([CH, 512], F32, tag="hy0")
                nc.tensor.matmul(hy0[:], ones_row[:], biascat[:, 0:512],
                                 start=True, stop=False)
                nc.tensor.matmul(hy0[:], stT[:], wcat[:, 0:512],
                                 start=False, stop=True)
                hy1 = psh.tile([CH, T], F32, tag="hy1")
                nc.tensor.matmul(hy1[:], ones_row[:], biascat[:, 512:NW],
                                 start=True, stop=False)
                nc.tensor.matmul(hy1[:], stT[:], wcat[:, 512:NW],
                                 start=False, stop=True)

                hsb = smp.tile([CH, 512], F32, tag="hsb")
                nc.scalar.copy(hsb[:], hy0[:])
                v_sb = smp.tile([CH, T], F32, tag="v")
                nc.scalar.copy(v_sb[:], hy1[:])

                # c = sum_e |wq*wk| / sqrt(e);  r = sum_e |wv*wo|
                pqk = smp.tile([CH, 128], F32, tag="pqk")
                nc.vector.tensor_tensor(pqk[:], hsb[:, 0:128],
                                        hsb[:, 128:256],
                                        op=mybir.AluOpType.mult)
                c0 = smp.tile([CH, 1], F32, tag="c0")
                nc.vector.tensor_reduce(c0[:], pqk[:], axis=mybir.AxisListType.X,
                                        op=mybir.AluOpType.add,
                                        apply_absolute_value=True)
                pvo = smp.tile([CH, 128], F32, tag="pvo")
                nc.vector.tensor_tensor(pvo[:], hsb[:, 256:384],
                                        hsb[:, 384:512],
                                        op=mybir.AluOpType.mult)
                r_sb = smp.tile([CH, 1], F32, tag="r")
                nc.vector.tensor_reduce(r_sb[:], pvo[:], axis=mybir.AxisListType.X,
                                        op=mybir.AluOpType.add,
                                        apply_absolute_value=True)

                # cx[b, k] = c_b/sqrt(e) * x[b, k]
                cx = smp.tile([CH, T], F32, tag="cx")
                nc.vector.tensor_scalar(cx[:], x_sb[:], c0[:], RSQRT_E,
                                        op0=mybir.AluOpType.mult,
                                        op1=mybir.AluOpType.mult)

                # L[b, q, k] = x[b, q] * cx[b, k]
                L = bigp.tile([CH, T, T], F32, tag="L")
                xq_b = x_sb[:].unsqueeze(2).broadcast_to([CH, T, T])
                cxk_b = cx[:].unsqueeze(1).broadcast_to([CH, T, T])
                nc.vector.tensor_tensor(L[:], xq_b, cxk_b,
                                        op=mybir.AluOpType.mult)

                # E = exp(L - C_SHIFT)
                E = bigp.tile([CH, T, T], F32, tag="E")
                nc.scalar.activation(E[:], L[:],
                                     mybir.ActivationFunctionType.Exp,
                                     bias=shift[:], scale=1.0)

                # EX = E * x_k ; numer/denom = segmented sums over k
                EX = bigp.tile([CH, T, T], F32, tag="EX")
                xk_b = x_sb[:].unsqueeze(1).broadcast_to([CH, T, T])
                nc.vector.tensor_tensor(EX[:], E[:], xk_b,
                                        op=mybir.AluOpType.mult)
                numer = smp.tile([CH, T], F32, tag="numer")
                nc.vector.tensor_reduce(numer[:], EX[:],
                                        axis=mybir.AxisListType.X,
                                        op=mybir.AluOpType.add)
                denom = smp.tile([CH, T], F32, tag="denom")
                nc.vector.tensor_reduce(denom[:], E[:],
                                        axis=mybir.AxisListType.X,
                                        op=mybir.AluOpType.add)

                # z = (numer/denom) * r + v ; out = elu(z)
                dinv = smp.tile([CH, T], F32, tag="dinv")
                nc.vector.reciprocal(dinv[:], denom[:])
                m_sb = smp.tile([CH, T], F32, tag="m")
                nc.vector.tensor_tensor(m_sb[:], numer[:], dinv[:],
                                        op=mybir.AluOpType.mult)
                z = smp.tile([CH, T], F32, tag="z")
                nc.vector.tensor_scalar(z[:], m_sb[:], r_sb[:], None,
                                        op0=mybir.AluOpType.mult)
                z2 = smp.tile([CH, T], F32, tag="z2")
                nc.vector.tensor_tensor(z2[:], z[:], v_sb[:],
                                        op=mybir.AluOpType.add)
                zn = smp.tile([CH, T], F32, tag="zn")
                nc.vector.tensor_scalar(zn[:], z2[:], 0.0, None,
                                        op0=mybir.AluOpType.min)
                ez = smp.tile([CH, T], F32, tag="ez")
                nc.scalar.activation(ez[:], zn[:],
                                     mybir.ActivationFunctionType.Exp)
                zp = smp.tile([CH, T], F32, tag="zp")
                nc.vector.tensor_scalar(zp[:], z2[:], 0.0, None,
                                        op0=mybir.AluOpType.max)
                s1 = smp.tile([CH, T], F32, tag="s1")
                nc.vector.tensor_tensor(s1[:], zp[:], ez[:],
                                        op=mybir.AluOpType.add)
                o_sb = smp.tile([CH, T], F32, tag="o")
                nc.vector.tensor_scalar(o_sb[:], s1[:], -1.0, None,
                                        op0=mybir.AluOpType.add)
                nc.sync.dma_start(od[bs:bs + CH, :], o_sb[:])

    nc.compile()
    return nc


def kernel(**inputs):
    nc = _compiled.get("nc")
    if nc is None:
        nc = _compiled["nc"] = _build()

    x = np.ascontiguousarray(np.asarray(inputs["x"], dtype=np.float32)
                             .reshape(B_FULL, T))
    state = np.ascontiguousarray(np.asarray(inputs["state"], dtype=np.float32))
    wcatT = np.ascontiguousarray(np.concatenate(
        [np.asarray(inputs["wk_w"], np.float32).T,
         np.asarray(inputs["wq_w"], np.float32).T,
         np.asarray(inputs["wv_w"], np.float32).T,
         np.asarray(inputs["wo_w"], np.float32).T,
         np.asarray(inputs["V_w"], np.float32).T], axis=1))
    biascat = np.ascontiguousarray(np.concatenate(
        [np.asarray(inputs["wk_b"], np.float32),
         np.asarray(inputs["wq_b"], np.float32),
         np.asarray(inputs["wv_b"], np.float32),
         np.asarray(inputs["wo_b"], np.float32),
         np.asarray(inputs["V_b"], np.float32)])[None, :])

    in_maps = []
    for i in range(N_CORES):
        sl = slice(i * BC, (i + 1) * BC)
        in_maps.append({
            "x": np.ascontiguousarray(x[sl]),
            "state": np.ascontiguousarray(state[sl]),
            "wcatT": wcatT,
            "biascat": biascat,
        })

    res = run_bass_kernel_spmd(nc, in_maps, core_ids=list(range(N_CORES)))
    out = np.concatenate([res.results[i]["out"] for i in range(N_CORES)],
                         axis=0)
    return out.reshape(B_FULL, 1, T)



# revision 4
# speedup vs baseline: 1.2928x; 1.2928x over previous
"""Trainium2 Bass kernel for nn_Attention_62715112456978.

The reference attention is algebraically rank-1: keys/queries/values are
outer products x ⊗ w, so

    dot[b,q,k]   = c_b * x[b,q] * x[b,k],   c_b = sum_e wq*wk / sqrt(e)
    softmax-out  = m[b,q] * wv[b,:],        m[b,q] = sum_k A[b,q,k]*x[b,k]
    final        = elu(m[b,q] * r_b + v[b,q]),  r_b = sum_e wv*wo

with wq/wk/wv/wo = |state @ W.T + b| (only the products c, r are needed,
and |a|*|b| = |a*b|, so the abs never has to be materialized).

Sharding: pure data parallel over batch; 8 cores x 512 batches each.
Weights (four 128x128 + V) are tiny and replicated to every core.
"""

import numpy as np

import concourse.bacc as bacc
import concourse.bass as bass
import concourse.tile as tile
from concourse import mybir
from concourse.bass_utils import run_bass_kernel_spmd

F32 = mybir.dt.float32
BF16 = mybir.dt.bfloat16

N_CORES = 8
B_FULL = 4096
BC = B_FULL // N_CORES  # 512 batches per core
CH = 128                # batches per chunk (partition dim)
NCH = BC // CH          # 4 chunks per core
T = 64                  # sequence length
D = 128                 # d_state
NW = 5 * 128 - 64       # 576 = wk|wq|wv|wo (128 each) + V (64) output columns
RSQRT_E = float(1.0 / np.sqrt(128.0))
C_SHIFT = 40.0          # global exp shift; cancels in softmax, avoids overflow

_compiled = {}


def _build():
    nc = bacc.Bacc("TRN2", target_bir_lowering=False, debug=False,
                   num_devices=N_CORES)
    xd = nc.dram_tensor("x", [BC, T], F32, kind="ExternalInput")
    sd = nc.dram_tensor("state", [BC, D], F32, kind="ExternalInput")
    wd = nc.dram_tensor("wcatT", [D, NW], F32, kind="ExternalInput")
    bd = nc.dram_tensor("biascat", [1, NW], F32, kind="ExternalInput")
    od = nc.dram_tensor("out", [BC, T], F32, kind="ExternalOutput")

    with tile.TileContext(nc) as tc:
        with (
            tc.tile_pool(name="const", bufs=1) as cpool,
            tc.tile_pool(name="io", bufs=2) as iop,
            tc.tile_pool(name="big", bufs=2) as bigp,
            tc.tile_pool(name="tree", bufs=2) as treep,
            tc.tile_pool(name="small", bufs=2) as smp,
            tc.tile_pool(name="psum", bufs=2, space="PSUM") as psp,
            tc.tile_pool(name="psum_h", bufs=2, space="PSUM") as psh,
        ):
            # one-time constants
            wcat = cpool.tile([D, NW], F32)
            nc.sync.dma_start(wcat[:], wd[:])
            biascat = cpool.tile([1, NW], F32)
            nc.sync.dma_start(biascat[:], bd[:])
            ones_row = cpool.tile([1, CH], F32)
            nc.gpsimd.memset(ones_row[:], 1.0)
            # identity for PE transpose
            col_i = cpool.tile([128, 128], mybir.dt.int32)
            nc.gpsimd.iota(col_i[:], pattern=[[1, 128]], base=0,
                           channel_multiplier=0)
            row_i = cpool.tile([128, 128], mybir.dt.int32)
            nc.gpsimd.iota(row_i[:], pattern=[[0, 128]], base=0,
                           channel_multiplier=1)
            ident = cpool.tile([128, 128], F32)
            nc.vector.tensor_tensor(ident[:], col_i[:], row_i[:],
                                    op=mybir.AluOpType.is_equal)
            shift = cpool.tile([128, 1], F32)
            nc.gpsimd.memset(shift[:], -C_SHIFT)

            for ci in range(NCH):
                bs = ci * CH
                x_sb = iop.tile([CH, T], F32, tag="x")
                nc.sync.dma_start(x_sb[:], xd[bs:bs + CH, :])
                st_sb = iop.tile([CH, D], F32, tag="st")
                nc.sync.dma_start(st_sb[:], sd[bs:bs + CH, :])

                # stateT via PE transpose (for the hypernet matmuls)
                stT_ps = psp.tile([D, CH], F32, tag="stT")
                nc.tensor.transpose(stT_ps[:], st_sb[:], ident[:])
                stT = smp.tile([D, CH], F32, tag="stTsb")
                nc.scalar.copy(stT[:], stT_ps[:])

                # hypernet: out[b, e] = state @ W.T + bias  (bias prefilled
                # via a K=1 ones-matmul, weights accumulate on top)
                hy0 = psh.tile([CH, 512], F32, tag="hy0")
                nc.tensor.matmul(hy0[:], ones_row[:], biascat[:, 0:512],
                                 start=True, stop=False)
                nc.tensor.matmul(hy0[:], stT[:], wcat[:, 0:512],
                                 start=False, stop=True)
                hy1 = psh.tile([CH, T], F32, tag="hy1")
                nc.tensor.matmul(hy1[:], ones_row[:], biascat[:, 512:NW],
                                 start=True, stop=False)
                nc.tensor.matmul(hy1[:], stT[:], wcat[:, 512:NW],
                                 start=False, stop=True)

                hsb = smp.tile([CH, 512], F32, tag="hsb")
                nc.scalar.copy(hsb[:], hy0[:])
                v_sb = smp.tile([CH, T], F32, tag="v")
                nc.scalar.copy(v_sb[:], hy1[:])

                # c = sum_e |wq*wk| / sqrt(e);  r = sum_e |wv*wo|
                pqk = smp.tile([CH, 128], F32, tag="pqk")
                nc.vector.tensor_tensor(pqk[:], hsb[:, 0:128],
                                        hsb[:, 128:256],
                                        op=mybir.AluOpType.mult)
                c0 = smp.tile([CH, 1], F32, tag="c0")
                nc.vector.tensor_reduce(c0[:], pqk[:], axis=mybir.AxisListType.X,
                                        op=mybir.AluOpType.add,
                                        apply_absolute_value=True)
                pvo = smp.tile([CH, 128], F32, tag="pvo")
                nc.vector.tensor_tensor(pvo[:], hsb[:, 256:384],
                                        hsb[:, 384:512],
                                        op=mybir.AluOpType.mult)
                r_sb = smp.tile([CH, 1], F32, tag="r")
                nc.vector.tensor_reduce(r_sb[:], pvo[:], axis=mybir.AxisListType.X,
                                        op=mybir.AluOpType.add,
                                        apply_absolute_value=True)

                # cx[b, k] = c_b/sqrt(e) * x[b, k]
                cx = smp.tile([CH, T], F32, tag="cx")
                nc.vector.tensor_scalar(cx[:], x_sb[:], c0[:], RSQRT_E,
                                        op0=mybir.AluOpType.mult,
                                        op1=mybir.AluOpType.mult)

                # L[b, q, k] = x[b, q] * cx[b, k]  (fp32 TT, DVE 1x)
                L = bigp.tile([CH, T, T], F32, tag="L")
                xq_b = x_sb[:].unsqueeze(2).broadcast_to([CH, T, T])
                cxk_b = cx[:].unsqueeze(1).broadcast_to([CH, T, T])
                nc.vector.tensor_tensor(L[:], xq_b, cxk_b,
                                        op=mybir.AluOpType.mult)

                # E = exp(L - C_SHIFT) -> bf16 (downstream DVE ops get 2x)
                E = bigp.tile([CH, T, T], BF16, tag="E")
                nc.scalar.activation(E[:], L[:],
                                     mybir.ActivationFunctionType.Exp,
                                     bias=shift[:], scale=1.0)

                # EX = E * x_k  (bf16 TT, 2x: inner dim stride-1 on both)
                x_bf = smp.tile([CH, T], BF16, tag="xbf")
                nc.vector.tensor_copy(x_bf[:], x_sb[:])
                EX = bigp.tile([CH, T, T], BF16, tag="EX")
                xk_b = x_bf[:].unsqueeze(1).broadcast_to([CH, T, T])
                nc.vector.tensor_tensor(EX[:], E[:], xk_b,
                                        op=mybir.AluOpType.mult)

                # segmented sums over k via halving adds (bf16 2x mode)
                # instead of tensor_reduce (hard-capped at 1x).
                def tree_sum(src_tile, tag, out_f32):
                    cur = src_tile
                    h = T // 2
                    while h >= 2:
                        nxt = treep.tile([CH, T, h], BF16, tag=f"{tag}{h}")
                        nc.vector.tensor_tensor(nxt[:], cur[:, :, 0:h],
                                                cur[:, :, h:2 * h],
                                                op=mybir.AluOpType.add)
                        cur = nxt
                        h //= 2
                    nc.vector.tensor_tensor(out_f32[:].unsqueeze(2),
                                            cur[:, :, 0:1], cur[:, :, 1:2],
                                            op=mybir.AluOpType.add)

                denom = smp.tile([CH, T], F32, tag="denom")
                tree_sum(E, "dn", denom)
                numer = smp.tile([CH, T], F32, tag="nm")
                tree_sum(EX, "nm", numer)

                # z = (numer/denom) * r + v ; out = elu(z)
                dinv = smp.tile([CH, T], F32, tag="dinv")
                nc.vector.reciprocal(dinv[:], denom[:])
                m_sb = smp.tile([CH, T], F32, tag="m")
                nc.vector.tensor_tensor(m_sb[:], numer[:], dinv[:],
                                        op=mybir.AluOpType.mult)
                z2 = smp.tile([CH, T], F32, tag="z2")
                nc.vector.scalar_tensor_tensor(z2[:], m_sb[:], r_sb[:],
                                               v_sb[:],
                                               op0=mybir.AluOpType.mult,
                                               op1=mybir.AluOpType.add)
                zn = smp.tile([CH, T], F32, tag="zn")
                nc.vector.tensor_scalar(zn[:], z2[:], 0.0, None,
                                        op0=mybir.AluOpType.min)
                ez = smp.tile([CH, T], F32, tag="ez")
                nc.scalar.activation(ez[:], zn[:],
                                     mybir.ActivationFunctionType.Exp)
                zp1 = smp.tile([CH, T], F32, tag="zp1")
                nc.vector.tensor_scalar(zp1[:], z2[:], 0.0, -1.0,
                                        op0=mybir.AluOpType.max,
                                        op1=mybir.AluOpType.add)
                o_sb = smp.tile([CH, T], F32, tag="o")
                nc.vector.tensor_tensor(o_sb[:], zp1[:], ez[:],
                                        op=mybir.AluOpType.add)
                nc.sync.dma_start(od[bs:bs + CH, :], o_sb[:])

    nc.compile()
    return nc


def kernel(**inputs):
    nc = _compiled.get("nc")
    if nc is None:
        nc = _compiled["nc"] = _build()

    x = np.ascontiguousarray(np.asarray(inputs["x"], dtype=np.float32)
                             .reshape(B_FULL, T))
    state = np.ascontiguousarray(np.asarray(inputs["state"], dtype=np.float32))
    wcatT = np.ascontiguousarray(np.concatenate(
        [np.asarray(inputs["wk_w"], np.float32).T,
         np.asarray(inputs["wq_w"], np.float32).T,
         np.asarray(inputs["wv_w"], np.float32).T,
         np.asarray(inputs["wo_w"], np.float32).T,
         np.asarray(inputs["V_w"], np.float32).T], axis=1))
    biascat = np.ascontiguousarray(np.concatenate(
        [np.asarray(inputs["wk_b"], np.float32),
         np.asarray(inputs["wq_b"], np.float32),
         np.asarray(inputs["wv_b"], np.float32),
         np.asarray(inputs["wo_b"], np.float32),
         np.asarray(inputs["V_b"], np.float32)])[None, :])

    in_maps = []
    for i in range(N_CORES):
        sl = slice(i * BC, (i + 1) * BC)
        in_maps.append({
            "x": np.ascontiguousarray(x[sl]),
            "state": np.ascontiguousarray(state[sl]),
            "wcatT": wcatT,
            "biascat": biascat,
        })

    res = run_bass_kernel_spmd(nc, in_maps, core_ids=list(range(N_CORES)))
    out = np.concatenate([res.results[i]["out"] for i in range(N_CORES)],
                         axis=0)
    return out.reshape(B_FULL, 1, T)



# revision 6
# speedup vs baseline: 1.2957x; 1.0023x over previous
"""Trainium2 Bass kernel for nn_Attention_62715112456978.

The reference attention is algebraically rank-1: keys/queries/values are
outer products x ⊗ w, so

    dot[b,q,k]   = c_b * x[b,q] * x[b,k],   c_b = sum_e wq*wk / sqrt(e)
    softmax-out  = m[b,q] * wv[b,:],        m[b,q] = sum_k A[b,q,k]*x[b,k]
    final        = elu(m[b,q] * r_b + v[b,q]),  r_b = sum_e wv*wo

with wq/wk/wv/wo = |state @ W.T + b| (only the products c, r are needed,
and |a|*|b| = |a*b|, so the abs never has to be materialized).

Sharding: pure data parallel over batch; 8 cores x 512 batches each.
Weights (four 128x128 + V) are tiny and replicated to every core.
"""

import numpy as np

import concourse.bacc as bacc
import concourse.bass as bass
import concourse.tile as tile
from concourse import mybir
from concourse.bass_utils import run_bass_kernel_spmd

F32 = mybir.dt.float32
BF16 = mybir.dt.bfloat16

N_CORES = 8
B_FULL = 4096
BC = B_FULL // N_CORES  # 512 batches per core
CH = 128                # batches per chunk (partition dim)
NCH = BC // CH          # 4 chunks per core
T = 64                  # sequence length
D = 128                 # d_state
NW = 5 * 128 - 64       # 576 = wk|wq|wv|wo (128 each) + V (64) output columns
RSQRT_E = float(1.0 / np.sqrt(128.0))
C_SHIFT = 40.0          # global exp shift; cancels in softmax, avoids overflow

_compiled = {}


def _build():
    nc = bacc.Bacc("TRN2", target_bir_lowering=False, debug=False,
                   num_devices=N_CORES)
    xd = nc.dram_tensor("x", [BC, T], F32, kind="ExternalInput")
    sd = nc.dram_tensor("state", [BC, D], F32, kind="ExternalInput")
    wd = nc.dram_tensor("wcatT", [D, NW], F32, kind="ExternalInput")
    bd = nc.dram_tensor("biascat", [1, NW], F32, kind="ExternalInput")
    od = nc.dram_tensor("out", [BC, T], F32, kind="ExternalOutput")

    with tile.TileContext(nc) as tc:
        with (
            tc.tile_pool(name="const", bufs=1) as cpool,
            tc.tile_pool(name="io", bufs=2) as iop,
            tc.tile_pool(name="big", bufs=2) as bigp,
            tc.tile_pool(name="tree", bufs=2) as treep,
            tc.tile_pool(name="small", bufs=2) as smp,
            tc.tile_pool(name="psum", bufs=2, space="PSUM") as psp,
            tc.tile_pool(name="psum_h", bufs=2, space="PSUM") as psh,
        ):
            # one-time constants
            wcat = cpool.tile([D, NW], F32)
            nc.sync.dma_start(wcat[:], wd[:])
            biascat = cpool.tile([1, NW], F32)
            nc.sync.dma_start(biascat[:], bd[:])
            ones_row = cpool.tile([1, CH], F32)
            nc.gpsimd.memset(ones_row[:], 1.0)
            # identity for PE transpose
            col_i = cpool.tile([128, 128], mybir.dt.int32)
            nc.gpsimd.iota(col_i[:], pattern=[[1, 128]], base=0,
                           channel_multiplier=0)
            row_i = cpool.tile([128, 128], mybir.dt.int32)
            nc.gpsimd.iota(row_i[:], pattern=[[0, 128]], base=0,
                           channel_multiplier=1)
            ident = cpool.tile([128, 128], F32)
            nc.vector.tensor_tensor(ident[:], col_i[:], row_i[:],
                                    op=mybir.AluOpType.is_equal)
            shift = cpool.tile([128, 1], F32)
            nc.gpsimd.memset(shift[:], -C_SHIFT)

            for ci in range(NCH):
                bs = ci * CH
                x_sb = iop.tile([CH, T], F32, tag="x")
                nc.sync.dma_start(x_sb[:], xd[bs:bs + CH, :])
                st_sb = iop.tile([CH, D], F32, tag="st")
                nc.sync.dma_start(st_sb[:], sd[bs:bs + CH, :])

                # stateT via PE transpose (for the hypernet matmuls)
                stT_ps = psp.tile([D, CH], F32, tag="stT")
                nc.tensor.transpose(stT_ps[:], st_sb[:], ident[:])
                stT = smp.tile([D, CH], F32, tag="stTsb")
                nc.scalar.copy(stT[:], stT_ps[:])

                # hypernet: out[b, e] = state @ W.T + bias  (bias prefilled
                # via a K=1 ones-matmul, weights accumulate on top)
                hy0 = psh.tile([CH, 512], F32, tag="hy0")
                nc.tensor.matmul(hy0[:], ones_row[:], biascat[:, 0:512],
                                 start=True, stop=False)
                nc.tensor.matmul(hy0[:], stT[:], wcat[:, 0:512],
                                 start=False, stop=True)
                hy1 = psh.tile([CH, T], F32, tag="hy1")
                nc.tensor.matmul(hy1[:], ones_row[:], biascat[:, 512:NW],
                                 start=True, stop=False)
                nc.tensor.matmul(hy1[:], stT[:], wcat[:, 512:NW],
                                 start=False, stop=True)

                hsb = smp.tile([CH, 512], F32, tag="hsb")
                nc.scalar.copy(hsb[:], hy0[:])
                v_sb = smp.tile([CH, T], F32, tag="v")
                nc.scalar.copy(v_sb[:], hy1[:])

                # c = sum_e |wq*wk| / sqrt(e);  r = sum_e |wv*wo|
                pqk = smp.tile([CH, 128], F32, tag="pqk")
                nc.vector.tensor_tensor(pqk[:], hsb[:, 0:128],
                                        hsb[:, 128:256],
                                        op=mybir.AluOpType.mult)
                c0 = smp.tile([CH, 1], F32, tag="c0")
                nc.vector.tensor_reduce(c0[:], pqk[:], axis=mybir.AxisListType.X,
                                        op=mybir.AluOpType.add,
                                        apply_absolute_value=True)
                pvo = smp.tile([CH, 128], F32, tag="pvo")
                nc.vector.tensor_tensor(pvo[:], hsb[:, 256:384],
                                        hsb[:, 384:512],
                                        op=mybir.AluOpType.mult)
                r_sb = smp.tile([CH, 1], F32, tag="r")
                nc.vector.tensor_reduce(r_sb[:], pvo[:], axis=mybir.AxisListType.X,
                                        op=mybir.AluOpType.add,
                                        apply_absolute_value=True)

                # cx[b, k] = c_b/sqrt(e) * x[b, k]
                cx = smp.tile([CH, T], F32, tag="cx")
                nc.vector.tensor_scalar(cx[:], x_sb[:], c0[:], RSQRT_E,
                                        op0=mybir.AluOpType.mult,
                                        op1=mybir.AluOpType.mult)

                # L[b, q, k] = x[b, q] * cx[b, k]  (fp32 TT, DVE 1x)
                L = bigp.tile([CH, T, T], F32, tag="L")
                xq_b = x_sb[:].unsqueeze(2).broadcast_to([CH, T, T])
                cxk_b = cx[:].unsqueeze(1).broadcast_to([CH, T, T])
                nc.vector.tensor_tensor(L[:], xq_b, cxk_b,
                                        op=mybir.AluOpType.mult)

                # E = exp(L - C_SHIFT) -> bf16 (downstream DVE ops get 2x)
                E = bigp.tile([CH, T, T], BF16, tag="E")
                nc.scalar.activation(E[:], L[:],
                                     mybir.ActivationFunctionType.Exp,
                                     bias=shift[:], scale=1.0)

                # EX = E * x_k  (bf16 TT, 2x: inner dim stride-1 on both)
                x_bf = smp.tile([CH, T], BF16, tag="xbf")
                nc.vector.tensor_copy(x_bf[:], x_sb[:])
                EX = bigp.tile([CH, T, T], BF16, tag="EX")
                xk_b = x_bf[:].unsqueeze(1).broadcast_to([CH, T, T])
                nc.vector.tensor_tensor(EX[:], E[:], xk_b,
                                        op=mybir.AluOpType.mult)

                # segmented sums over k: halving adds in bf16 (2x mode) down
                # to width 8, then one short tensor_reduce tail (1x but small)
                def tree_sum(src_tile, tag, out_f32):
                    cur = src_tile
                    h = T // 2
                    while h >= 8:
                        nxt = treep.tile([CH, T, h], BF16, tag=f"{tag}{h}")
                        nc.vector.tensor_tensor(nxt[:], cur[:, :, 0:h],
                                                cur[:, :, h:2 * h],
                                                op=mybir.AluOpType.add)
                        cur = nxt
                        h //= 2
                    nc.vector.tensor_reduce(out_f32[:], cur[:],
                                            axis=mybir.AxisListType.X,
                                            op=mybir.AluOpType.add)

                denom = smp.tile([CH, T], F32, tag="denom")
                tree_sum(E, "dn", denom)
                numer = smp.tile([CH, T], F32, tag="nm")
                tree_sum(EX, "nm", numer)

                # z = (numer/denom) * r + v ; out = elu(z)
                dinv = smp.tile([CH, T], F32, tag="dinv")
                nc.vector.reciprocal_approx_fast(dinv[:], denom[:])
                m_sb = smp.tile([CH, T], F32, tag="m")
                nc.vector.tensor_tensor(m_sb[:], numer[:], dinv[:],
                                        op=mybir.AluOpType.mult)
                z2 = smp.tile([CH, T], F32, tag="z2")
                nc.vector.scalar_tensor_tensor(z2[:], m_sb[:], r_sb[:],
                                               v_sb[:],
                                               op0=mybir.AluOpType.mult,
                                               op1=mybir.AluOpType.add)
                zn = smp.tile([CH, T], F32, tag="zn")
                nc.vector.tensor_scalar(zn[:], z2[:], 0.0, None,
                                        op0=mybir.AluOpType.min)
                ez = smp.tile([CH, T], F32, tag="ez")
                nc.scalar.activation(ez[:], zn[:],
                                     mybir.ActivationFunctionType.Exp)
                zp1 = smp.tile([CH, T], F32, tag="zp1")
                nc.vector.tensor_scalar(zp1[:], z2[:], 0.0, -1.0,
                                        op0=mybir.AluOpType.max,
                                        op1=mybir.AluOpType.add)
                o_sb = smp.tile([CH, T], F32, tag="o")
                nc.vector.tensor_tensor(o_sb[:], zp1[:], ez[:],
                                        op=mybir.AluOpType.add)
                nc.sync.dma_start(od[bs:bs + CH, :], o_sb[:])

    nc.compile()
    return nc


def kernel(**inputs):
    nc = _compiled.get("nc")
    if nc is None:
        nc = _compiled["nc"] = _build()

    x = np.ascontiguousarray(np.asarray(inputs["x"], dtype=np.float32)
                             .reshape(B_FULL, T))
    state = np.ascontiguousarray(np.asarray(inputs["state"], dtype=np.float32))
    wcatT = np.ascontiguousarray(np.concatenate(
        [np.asarray(inputs["wk_w"], np.float32).T,
         np.asarray(inputs["wq_w"], np.float32).T,
         np.asarray(inputs["wv_w"], np.float32).T,
         np.asarray(inputs["wo_w"], np.float32).T,
         np.asarray(inputs["V_w"], np.float32).T], axis=1))
    biascat = np.ascontiguousarray(np.concatenate(
        [np.asarray(inputs["wk_b"], np.float32),
         np.asarray(inputs["wq_b"], np.float32),
         np.asarray(inputs["wv_b"], np.float32),
         np.asarray(inputs["wo_b"], np.float32),
         np.asarray(inputs["V_b"], np.float32)])[None, :])

    in_maps = []
    for i in range(N_CORES):
        sl = slice(i * BC, (i + 1) * BC)
        in_maps.append({
            "x": np.ascontiguousarray(x[sl]),
            "state": np.ascontiguousarray(state[sl]),
            "wcatT": wcatT,
            "biascat": biascat,
        })

    res = run_bass_kernel_spmd(nc, in_maps, core_ids=list(range(N_CORES)))
    out = np.concatenate([res.results[i]["out"] for i in range(N_CORES)],
                         axis=0)
    return out.reshape(B_FULL, 1, T)

